# revision 17
# baseline (speedup 1.0000x reference)
"""DRN layer kernel for 8 TRN2 NeuronCores (4-way group-sum + fp8 DoubleRow).

Math (reference):
    T[j,k,l,m]   = exp(-w[j,k] * (s0[m]-s1[l])^2)
    Pw[i,j,k,l]  = sum_m T[j,k,l,m] * P[i,k,m]
    logsum[i,j,l]= sum_k log(Pw[i,j,k,l])
    out          = softmax_l(logsum + exponent_B[j,l])

With P' = P/S and t' = T - 1:  log Pw = log S + log1p(r),
r = sum_m t' P', |r| <= 0.105. log S cancels in the softmax.

Group-sum approximation: sum_{k in G} log1p(r_k) ~= log1p(sum r_k)
for groups of 4 ks. The dropped cross terms are minimized by a greedy
weights-only matching (group ks whose w[:,a]*w[:,b] sums are most
negative); measured softmax error ~6e-3 + ~4e-3 of fp8 quantization
against the 2e-2 tolerance. Each group's R = sum of 4 r_k comes out of
ONE fp8 DoubleRow matmul (256 contraction slots = 4x64 m-rows packed
2-per-cell): 32 MMs per core instead of 128. The PE is throttled to
1.2 GHz on this part (427ns per N=512 matmul), so the PE floor drops
4x to ~14us and the PSUM drain work drops with it.

Sharding: tensor-parallel over n_upper: 8 cores x 8 upper nodes, full
batch per core. 16 group-tiles of R land in PSUM (fp32); each is
consumed once by one of three routes:
  c) DVE fused chain   chain = (R + 1) * chain   (scalar_tensor_tensor)
  g) ScalarE log1p(R) -> f32, GpSimd adds into an SBUF accumulator
  d) ScalarE log1p(R) -> f32, DMA inline-accumulate (SWDGE CCE add)
Final: E = exp(acc_g + acc_d) * chain0 * chain1, then sum_l, normalize.
"""

import numpy as np

B, NU, NL, QU, QL = 256, 64, 64, 64, 64
NCORES = 8
JLOC = NU // NCORES  # 8 upper nodes per core
JL = JLOC * QU       # 512 = packed (j, l) free dim
NGRP = NL // 4       # 16 k-groups of 4
KP2 = 128            # DoubleRow: 256 contraction slots as [128 part, 2]
PWK = B + JL         # 768 packed width per group: [P'^T (256 i) | t' (512)]
NKB = NGRP // 2      # 8 two-group DMA blocks


# route per pair-tile: c0/c1 = DVE product chains, g = ScalarE log +
# GpSimd accumulate, d = ScalarE log + DMA inline-accumulate. g/d end
# early (Q7 pipeline drain + SDMA completion lag); tail is chains.
def _make_route():
    route = [None] * NGRP
    dpos = [2, 6]
    gpos = [1, 4, 8, 11]
    ci = 0
    for p in range(NGRP):
        if p in dpos:
            route[p] = "d"
        elif p in gpos:
            route[p] = "g"
        else:
            route[p] = f"c{ci}"
            ci ^= 1
    return route


ROUTE = _make_route()
assert len(ROUTE) == NGRP


def _build_program():
    import concourse.bass as bass
    import concourse.bacc as bacc
    import concourse.mybir as mybir
    from concourse.tile import TileContext

    f32 = mybir.dt.float32
    bf16 = mybir.dt.bfloat16
    AF = mybir.ActivationFunctionType
    ALU = mybir.AluOpType

    nc = bacc.Bacc(None, target_bir_lowering=False)
    fp8 = mybir.dt.float8e4
    PTT = nc.declare_dram_parameter("PTT", [NKB, KP2, 4 * PWK], fp8,
                                    isOutput=False)
    EB = nc.declare_dram_parameter("EB", [128, 2 * JL], f32, isOutput=False)
    OUT = nc.declare_dram_parameter("out", [2, 128, JL], f32, isOutput=True)

    with TileContext(nc) as tc:
        with (
            tc.tile_pool(name="ptt", bufs=5) as ppool,
            tc.tile_pool(name="cst", bufs=1) as cpool,
            tc.tile_pool(name="ps", bufs=4, space="PSUM") as pspool,
            tc.tile_pool(name="lgf", bufs=3) as lfpool,
            tc.tile_pool(name="ch", bufs=1) as chpool,
            tc.tile_pool(name="sm", bufs=2) as smpool,
            tc.tile_pool(name="ot", bufs=2) as opool,
        ):
            ebt = cpool.tile([128, 2 * JL], f32, tag="ebt")

            acc_g = chpool.tile([128, 2 * JL], f32, tag="accg", name="accg")
            acc_d = chpool.tile([128, 2 * JL], f32, tag="accd", name="accd")
            nc.vector.memset(acc_d[:], 0.0)
            chains = {
                "c0": chpool.tile([128, 2 * JL], f32, tag="ch0", name="ch0"),
                "c1": chpool.tile([128, 2 * JL], f32, tag="ch1", name="ch1"),
            }
            started = {"c0": False, "c1": False, "g": False}

            for kb in range(NKB):
                ptt = ppool.tile([KP2, 4 * PWK], fp8, tag="ptt")
                dge = nc.sync if kb % 2 == 0 else nc.scalar
                dge.dma_start(out=ptt[:], in_=PTT[kb])
                if kb == 0:
                    nc.sync.dma_start(out=ebt[:], in_=EB[:, :])
                for kk in range(2):
                    p = 2 * kb + kk
                    r = ROUTE[p]
                    pk = ptt[:, 2 * kk * PWK:2 * (kk + 1) * PWK].rearrange(
                        "q (c w) -> q c w", c=2)
                    ps = pspool.tile([128, 2 * JL], f32, tag="ps", name="ps")
                    for ih in range(2):
                        nc.tensor.matmul(
                            ps[:, ih * JL:(ih + 1) * JL],
                            lhsT=pk[:, :, ih * 128:(ih + 1) * 128],
                            rhs=pk[:, :, B:PWK],
                            start=True, stop=True,
                            perf_mode=mybir.MatmulPerfMode.DoubleRow)
                    if r in ("c0", "c1"):
                        ch = chains[r]
                        if not started[r]:
                            nc.vector.tensor_scalar_add(ch[:], ps[:], 1.0)
                            started[r] = True
                        else:
                            nc.vector.scalar_tensor_tensor(
                                ch[:], ps[:], 1.0, ch[:],
                                op0=ALU.add, op1=ALU.mult)
                    else:  # g / d: log1p then accumulate off the DVE
                        lgf = lfpool.tile([128, 2 * JL], f32, tag="lgf",
                                          name="lgf")
                        nc.scalar.activation(lgf[:], ps[:], AF.Ln, bias=1.0)
                        if r == "d":
                            nc.gpsimd.dma_start(
                                out=acc_d[:], in_=lgf[:], accum_op=ALU.add)
                        elif not started["g"]:
                            # first accumulate folds in exponent_B
                            nc.gpsimd.tensor_add(acc_g[:], ebt[:], lgf[:])
                            started["g"] = True
                        else:
                            nc.gpsimd.tensor_add(acc_g[:], acc_g[:], lgf[:])

            # tail, pipelined by ih-half across ScalarE/DVE:
            # E = exp(acc_g + acc_d) * chain0 * chain1; logits centered
            # (log S dropped) so no max-shift. Folds run on DVE: GpSimd's
            # last op carries a ~3us pipeline drain.
            NG = JLOC  # 8 j-groups per half
            exs = opool.tile([128, 2 * JL], f32, tag="exs")
            ot = opool.tile([128, 2 * JL], f32, tag="otb", name="otb")
            smb = smpool.tile([128, 2 * NG], f32, tag="smb")
            rcb = smpool.tile([128, 2 * NG], f32, tag="rcb")
            for ih in range(2):
                hs = slice(ih * JL, (ih + 1) * JL)
                gs = slice(ih * NG, (ih + 1) * NG)
                nc.vector.tensor_add(acc_g[:, hs], acc_g[:, hs],
                                     acc_d[:, hs])
                nc.scalar.activation(exs[:, hs], acc_g[:, hs], AF.Exp)
                nc.vector.tensor_mul(exs[:, hs], exs[:, hs],
                                     chains["c0"][:, hs])
                nc.vector.tensor_mul(exs[:, hs], exs[:, hs],
                                     chains["c1"][:, hs])
                exs3 = exs[:, hs].rearrange("p (g l) -> p g l", g=NG)
                nc.vector.tensor_reduce(
                    smb[:, gs], exs3, axis=mybir.AxisListType.X, op=ALU.add)
                nc.vector.reciprocal(rcb[:, gs], smb[:, gs])
                ot3 = ot[:, hs].rearrange("p (g l) -> p g l", g=NG)
                nc.vector.tensor_mul(
                    ot3, exs3, rcb[:, gs].broadcast_to((128, NG, QU)))
                nc.sync.dma_start(out=OUT[ih, :, :], in_=ot[:, hs])
    nc.compile()
    return nc


def _host_prep(P, weight, bias_abs, bias_q, lambda_abs, lambda_q):
    """Per-core input maps. Host does only O(weights) work plus linear
    passes over P (sum, normalize, transpose, cast)."""
    import ml_dtypes

    bf16 = ml_dtypes.bfloat16
    s1 = np.arange(QU, dtype=np.float64) / QU
    s0 = np.arange(QL, dtype=np.float64) / QL
    diff2 = (s0[None, :] - s1[:, None]) ** 2            # [l, m]
    t_full = np.expm1(-weight[:, :, None, None].astype(np.float64)
                      * diff2[None, None, :, :]).astype(np.float32)
    sq = s1
    expB = (-bias_q.astype(np.float64) * (sq[None, :] - lambda_q) ** 2
            - bias_abs.astype(np.float64)
            * np.abs(sq[None, :] - lambda_abs)).astype(np.float32)

    fp8 = ml_dtypes.float8_e4m3
    P32 = P.astype(np.float32)
    S = P32.sum(axis=2, dtype=np.float64)               # [i, k]
    Pn = (P32 / S[:, :, None]).astype(np.float32)       # P' = P/S
    PT_f8 = Pn.transpose(1, 2, 0).astype(fp8)           # [k, m, i]

    # Group assignment: the dropped 4-way cross term is
    # sum_{a<b in group} r_a r_b with r_k ~ -w[j,k] A_k, so greedily
    # group ks to make the in-group sums of w[:,a]*w[:,b] as negative
    # as possible (pairs first, then pair the pairs). Weights-only,
    # measured to cut the grouping error ~30%.
    M = weight.T @ weight                                # [NL, NL]
    un = list(range(NL))
    pairs = []
    while un:
        a = un.pop(0)
        b = min(un, key=lambda x: M[a, x])
        un.remove(b)
        pairs.append((a, b))
    unp = list(range(len(pairs)))
    perm = []
    while unp:
        p = unp.pop(0)
        a, b = pairs[p]
        q = min(unp, key=lambda q_: M[a, pairs[q_][0]] + M[a, pairs[q_][1]]
                + M[b, pairs[q_][0]] + M[b, pairs[q_][1]])
        unp.remove(q)
        perm.extend(pairs[p] + pairs[q])

    in_maps = []
    for c in range(NCORES):
        tc_ = t_full[c * JLOC:(c + 1) * JLOC]           # [8, k, l, m]
        tc_ = tc_.transpose(1, 3, 0, 2).reshape(NL, QL, JL)  # [k, m, (j,l)]
        # per k: [64 m, 768] rows [P'^T | t']; groups stack 4 ks into
        # 256 contraction slots s, packed DoubleRow-style as [kp, cc]
        # with s = 2*kp + cc; two groups per DMA block
        PTTk = np.empty((NL, QL, PWK), dtype=fp8)
        PTTk[:, :, :B] = PT_f8
        PTTk[:, :, B:] = tc_.astype(fp8)
        PTTk = PTTk[perm]                               # grouping order
        PTTg = PTTk.reshape(NGRP, 2 * KP2, PWK)         # [grp, s, row]
        PTTg = PTTg.reshape(NGRP, KP2, 2 * PWK)         # [grp, kp, cc*row]
        PTTc = np.ascontiguousarray(
            PTTg.reshape(NKB, 2, KP2, 2 * PWK).transpose(0, 2, 1, 3)
            .reshape(NKB, KP2, 4 * PWK))
        eb_row = np.tile(expB[c * JLOC:(c + 1) * JLOC].reshape(JL), 2)
        EBc = np.ascontiguousarray(
            np.broadcast_to(eb_row, (128, 2 * JL)).astype(np.float32))
        in_maps.append({"PTT": PTTc, "EB": EBc})
    return in_maps


_PROGRAM = None


def _get_program():
    global _PROGRAM
    if _PROGRAM is None:
        _PROGRAM = _build_program()
    return _PROGRAM


def run_on_device(in_maps, trace=False):
    from concourse.bass_utils import run_bass_kernel_spmd
    nc = _get_program()
    return run_bass_kernel_spmd(
        nc, in_maps, core_ids=list(range(NCORES)), trace=trace,
    )


def assemble(results):
    out = np.empty((B, NU, QU), dtype=np.float32)
    for c in range(NCORES):
        rc = results[c]["out"].reshape(B, JLOC, QU)
        out[:, c * JLOC:(c + 1) * JLOC, :] = rc
    return out


def kernel(P, weight, bias_abs, bias_q, lambda_abs, lambda_q):
    in_maps = _host_prep(P, weight, bias_abs, bias_q, lambda_abs, lambda_q)
    res = run_on_device(in_maps, trace=False)
    return assemble(res.results)
